# revision 1
# baseline (speedup 1.0000x reference)
"""GCN block kernel for Trainium2 (8 NeuronCores, SPMD).

Computes: h = A @ (x @ W) + b; BatchNorm1d(train, biased var); LeakyReLU(0.2)
  x: [16384, 128] f32, A: [16384, 16384] f32, W: [128, 128], b/gamma/beta: [128]

Strategy (row-shard over output nodes, 8 cores x 2048 rows):
  - Host passes A^T column-blocks per core (contraction dim on partitions),
    x^T (so XW tiles can be built with natural-layout PE matmuls), and the
    small params.
  - Each core computes XW = x @ W fully (replicated, cheap), then
    hT_shard[f, n] = sum_k XW[k, f] * AT[k, n] accumulated over 128 k-chunks
    in PSUM, streaming A^T tiles from HBM (memory-bound, ~134 MB/core).
  - BN stats = free-axis reductions in the hT layout; 8-core AllReduce of
    [128, 2] sums; affine+LeakyReLU fused in one ACT Prelu op; PE-transpose
    back to natural layout and DMA out.
  - A post-compile pass strips the redundant per-matmul LDWEIGHTS reloads
    within each same-weight group.
  - A dummy AllReduce early in the program absorbs the ~64 us cold ncfw
    trigger cost so the tail stats AllReduce starts in ~1 us.
Measured on 8 trn2 NeuronCores: 287-319 us (typ ~300), rel err 4.7e-4
(A_MODE="f16").
"""

import numpy as np

import concourse.bass as bass
import concourse.bacc as bacc
import concourse.mybir as mybir
import concourse.tile as tile
from concourse.bass_utils import run_bass_kernel_spmd

N = 16384
D = 128
NCORES = 8
R = N // NCORES          # 2048 rows per core
KCH = N // 128           # 128 k-chunks
EPS = 1e-5
NEG_SLOPE = 0.2

# Precision/speed mode for the big A @ XW matmul:
#   "f16"    - A and XW cast to fp16: half DMA bytes, full-rate PE, and 10
#              mantissa bits (end-to-end rel err ~5e-4; A is U[0,1] and x is
#              N(0,1), so fp16 range is ample and PSUM accumulates in fp32)
#   "bf16"   - same speed as f16 but only 7 mantissa bits (~4e-3)
#   "bf16x2" - A and XW split hi+lo bf16, 3 matmul terms (near-fp32 accuracy,
#              same DMA bytes as fp32, PE-bound)
#   "f32r"   - single-pass fp32 (tf32-ish accuracy, PE-bound, ~1.8x slower)
#   "f32"    - exact 2-pass fp32 (slowest)
A_MODE = "f16"
# LeakyReLU implementation: "prelu" (1 ACT op) or "max" (DVE max(z, 0.2z)).
# ACT Lrelu is unusable: its slope is hardcoded to 0.01 and ignores alpha.
APPLY_MODE = "prelu"

F32 = mybir.dt.float32
F32R = mybir.dt.float32r
BF16 = mybir.dt.bfloat16
F16 = mybir.dt.float16
# dtype of the A stream + XW operand buffer (walrus requires the producer
# chain of an FP32r matmul operand to be f32r-typed end to end)
AT_DT = {"bf16": BF16, "f16": F16, "bf16x2": BF16, "f32r": F32R,
         "f32": F32}[A_MODE]


def build_program(raw_h=False):
    nc = bacc.Bacc("TRN2", target_bir_lowering=False, debug=False,
                   num_devices=NCORES)

    at = nc.dram_tensor("at", [N, R], AT_DT, kind="ExternalInput")
    atlo = (nc.dram_tensor("atlo", [N, R], BF16, kind="ExternalInput")
            if A_MODE == "bf16x2" else None)
    # in pure-bf16 mode the XW matmuls also run in bf16 (host sends bf16 xT)
    XT_DT = AT_DT if A_MODE in ("bf16", "f16") else F32
    xt = nc.dram_tensor("xt", [D, N], XT_DT, kind="ExternalInput")
    w = nc.dram_tensor("w", [D, D], F32, kind="ExternalInput")
    bvec = nc.dram_tensor("bvec", [D, 1], F32, kind="ExternalInput")
    gam = nc.dram_tensor("gam", [D, 1], F32, kind="ExternalInput")
    bet = nc.dram_tensor("bet", [D, 1], F32, kind="ExternalInput")
    ident = nc.dram_tensor("ident", [D, D], F32, kind="ExternalInput")
    if raw_h:
        out = nc.dram_tensor("out", [D, R], F32, kind="ExternalOutput")
    else:
        out = nc.dram_tensor("out", [R, D], F32, kind="ExternalOutput")

    # k-chunks per DMA group, sized so each dma_start moves 1-2 MiB
    # (bf16x2 keeps CPD=2 so the hi+lo staging tiles fit in SBUF)
    CPD = 2 if A_MODE not in ("bf16", "f16") else 4
    NGRP = KCH // CPD

    with tile.TileContext(nc) as tc:
        with (
            tc.tile_pool(name="const", bufs=1) as cpool,
            tc.tile_pool(name="xw", bufs=1) as xwpool,
            tc.tile_pool(name="at", bufs=6) as atpool,
            tc.tile_pool(name="work", bufs=1) as wpool,
            tc.tile_pool(name="psum_h", bufs=1, space="PSUM") as ph,
            tc.tile_pool(name="psum_s", bufs=2, space="PSUM") as ps,
            tc.tile_pool(name="dram", bufs=1, space="DRAM") as dpool,
        ):
            # ---- constants / params ----
            w_sb = cpool.tile([D, D], F32)
            nc.sync.dma_start(w_sb[:], w[:])
            id_sb = cpool.tile([D, D], F32)
            nc.sync.dma_start(id_sb[:], ident[:])
            b_sb = cpool.tile([D, 1], F32)
            nc.sync.dma_start(b_sb[:], bvec[:])
            gam_sb = cpool.tile([D, 1], F32)
            nc.sync.dma_start(gam_sb[:], gam[:])
            bet_sb = cpool.tile([D, 1], F32)
            nc.sync.dma_start(bet_sb[:], bet[:])
            zero_sb = cpool.tile([D, 1], F32)
            nc.gpsimd.memset(zero_sb[:], 0.0)
            eps_sb = cpool.tile([D, 1], F32)
            nc.gpsimd.memset(eps_sb[:], EPS)

            # weights operand for the XW matmuls, in the xt dtype
    
            if XT_DT == F32:
                wmm_sb = w_sb
            else:
                wmm_sb = cpool.tile([D, D], XT_DT, name="wmm_sb")
                nc.vector.tensor_copy(wmm_sb[:], w_sb[:])

            # ---- phase 1: XW[k, f] tiles (node index on partitions) ----
            # XW_sb[:, t*128 + f] = XW[t*128 + p, f]
            # xT is loaded in 2048-node chunks so the XW matmuls (and the
            # first A-stream matmuls behind them) start almost immediately.
            xw_sb = xwpool.tile([D, N], AT_DT)
            xwlo_sb = (xwpool.tile([D, N], BF16, name="xwlo_sb")
                       if A_MODE == "bf16x2" else None)
            XTC = 16  # xw tiles per xt chunk
            for c in range(KCH // XTC):
                xtc = xwpool.tile([D, XTC * D], XT_DT, tag="xtc", bufs=3)
                nc.sync.dma_start(xtc[:], xt[:, bass.ts(c, XTC * D)])
                for g in range(XTC // 4):
                    # 4 XW tiles packed into one PSUM bank -> 1 wide copy
                    pxw = ps.tile([D, 4 * D], F32, tag="pxw", bufs=4)
                    for u in range(4):
                        nc.tensor.matmul(pxw[:, bass.ts(u, D)],
                                         xtc[:, bass.ts(4 * g + u, D)],
                                         wmm_sb[:], start=True, stop=True)
                    t0 = XTC * c + 4 * g
                    nc.vector.tensor_copy(xw_sb[:, bass.ts(t0 // 4, 4 * D)],
                                          pxw[:])
                    if xwlo_sb is not None:
                        # lo = bf16(XW - f32(bf16(XW)))
                        nc.vector.tensor_sub(
                            xwlo_sb[:, bass.ts(t0 // 4, 4 * D)], pxw[:],
                            xw_sb[:, bass.ts(t0 // 4, 4 * D)])

            # warm up the collective path early so the real stats
            # AllReduce at the tail doesn't pay the cold ncfw trigger cost
            if not raw_h:
                warm_in = dpool.tile([D, 2], F32, name="warm_in")
                warm_out = dpool.tile([D, 2], F32, addr_space="Shared",
                                      name="warm_out")
                warm_sb = cpool.tile([D, 2], F32, name="warm_sb")
                nc.gpsimd.memset(warm_sb[:], 0.0)
                nc.gpsimd.dma_start(warm_in[:], warm_sb[:])
                nc.gpsimd.collective_compute(
                    "AllReduce", mybir.AluOpType.add,
                    replica_groups=[list(range(NCORES))],
                    ins=[warm_in.opt()], outs=[warm_out.opt()])

            # ---- phase 2: hT[f, n] accumulation over k ----
            at_r = at.ap().rearrange("(c p) j -> c p j", p=128)
            atlo_r = (atlo.ap().rearrange("(c p) j -> c p j", p=128)
                      if atlo is not None else None)
            psum_h = ph.tile([D, R], F32)  # 4 PSUM banks
            for i in range(NGRP):
                at_t = atpool.tile([128, CPD, R], AT_DT, tag="at")
                dma_eng = nc.sync if i % 2 == 0 else nc.scalar
                dma_eng.dma_start(
                    at_t[:], at_r[CPD * i:CPD * (i + 1)].transpose([1, 0, 2]))
                if atlo_r is not None:
                    atlo_t = atpool.tile([128, CPD, R], BF16, tag="atlo")
                    nc.sync.dma_start(
                        atlo_t[:],
                        atlo_r[CPD * i:CPD * (i + 1)].transpose([1, 0, 2]))
                for a in range(CPD):
                    k = CPD * i + a
                    for s in range(R // 512):
                        nc.tensor.matmul(
                            psum_h[:, bass.ts(s, 512)],
                            xw_sb[:, bass.ts(k, D)],
                            at_t[:, a, bass.ts(s, 512)],
                            start=(k == 0), stop=(A_MODE != "bf16x2" and k == KCH - 1),
                        )
                    if A_MODE == "bf16x2":
                        # correction terms: hi-weights x lo-A, lo-weights x hi-A
                        for s in range(R // 512):
                            nc.tensor.matmul(
                                psum_h[:, bass.ts(s, 512)],
                                xw_sb[:, bass.ts(k, D)],
                                atlo_t[:, a, bass.ts(s, 512)],
                                start=False, stop=False)
                        for s in range(R // 512):
                            nc.tensor.matmul(
                                psum_h[:, bass.ts(s, 512)],
                                xwlo_sb[:, bass.ts(k, D)],
                                at_t[:, a, bass.ts(s, 512)],
                                start=False, stop=(k == KCH - 1))

            if raw_h:
                hraw = wpool.tile([D, R], F32)
                for s in range(4):
                    nc.scalar.activation(
                        hraw[:, bass.ts(s, 512)], psum_h[:, bass.ts(s, 512)],
                        mybir.ActivationFunctionType.Identity, bias=b_sb[:])
                nc.sync.dma_start(out.ap(), hraw[:])
            else:
                # ---- phase 3: bias add (PSUM->SBUF) + partial sums ----
                h_sb = wpool.tile([D, R], F32)
                sums = wpool.tile([D, 8], F32)
                for s in range(4):
                    nc.scalar.activation(
                        h_sb[:, bass.ts(s, 512)], psum_h[:, bass.ts(s, 512)],
                        mybir.ActivationFunctionType.Identity,
                        bias=b_sb[:], accum_out=sums[:, s:s + 1])
                sq_sb = atpool.tile([128, R], F32, tag="scr", bufs=3,
                                    name="sq_sb")
                for s in range(4):
                    nc.scalar.activation(
                        sq_sb[:, bass.ts(s, 512)], h_sb[:, bass.ts(s, 512)],
                        mybir.ActivationFunctionType.Square,
                        bias=zero_sb[:], accum_out=sums[:, 4 + s:5 + s])

                stats = wpool.tile([D, 2], F32)
                nc.vector.reduce_sum(stats[:, 0:1], sums[:, 0:4],
                                     axis=mybir.AxisListType.X)
                nc.vector.reduce_sum(stats[:, 1:2], sums[:, 4:8],
                                     axis=mybir.AxisListType.X)

                # ---- phase 4: AllReduce of [128, 2] stats across 8 cores ----
                cc_in = dpool.tile([D, 2], F32)
                cc_out = dpool.tile([D, 2], F32, addr_space="Shared")
                nc.gpsimd.dma_start(cc_in[:], stats[:])
                nc.gpsimd.collective_compute(
                    "AllReduce", mybir.AluOpType.add,
                    replica_groups=[list(range(NCORES))],
                    ins=[cc_in.opt()], outs=[cc_out.opt()])
                stats_g = wpool.tile([D, 2], F32)
                nc.gpsimd.dma_start(stats_g[:], cc_out[:])

                # ---- phase 5: per-feature scale/shift ----
                mean = wpool.tile([D, 1], F32)
                nc.scalar.mul(mean[:], stats_g[:, 0:1], 1.0 / N)
                ex2 = wpool.tile([D, 1], F32)
                nc.scalar.mul(ex2[:], stats_g[:, 1:2], 1.0 / N)
                msq = wpool.tile([D, 1], F32)
                nc.vector.tensor_mul(msq[:], mean[:], mean[:])
                var = wpool.tile([D, 1], F32)
                nc.vector.tensor_sub(var[:], ex2[:], msq[:])
                std = wpool.tile([D, 1], F32)
                nc.scalar.activation(std[:], var[:],
                                     mybir.ActivationFunctionType.Sqrt,
                                     bias=eps_sb[:])
                istd = wpool.tile([D, 1], F32)
                nc.vector.reciprocal(istd[:], std[:])
                scl = wpool.tile([D, 1], F32)
                nc.vector.tensor_mul(scl[:], gam_sb[:], istd[:])
                tmp = wpool.tile([D, 1], F32)
                nc.vector.tensor_mul(tmp[:], mean[:], scl[:])
                shf = wpool.tile([D, 1], F32)
                nc.vector.tensor_sub(shf[:], bet_sb[:], tmp[:])

                # ---- phase 6: y = LeakyReLU(scl*h + shf), still [f, n] ----
                hn = atpool.tile([128, R], F32, tag="scr", bufs=3,
                                 name="hn_sb")[:]
                if APPLY_MODE == "prelu":
                    for s in range(4):
                        nc.scalar.activation(
                            hn[:, bass.ts(s, 512)], h_sb[:, bass.ts(s, 512)],
                            mybir.ActivationFunctionType.Prelu,
                            bias=shf[:], scale=scl[:], alpha=NEG_SLOPE)
                else:
                    zt = sq_sb[:]
                    nc.scalar.activation(zt, h_sb[:],
                                         mybir.ActivationFunctionType.Identity,
                                         bias=shf[:], scale=scl[:])
                    nc.vector.tensor_scalar_mul(hn, zt, NEG_SLOPE)
                    nc.vector.tensor_max(hn, hn, zt)

                # ---- phase 7: transpose to [n, f] and store ----
                out_sb = atpool.tile([128, R], F32, tag="scr", bufs=3,
                                     name="out_t")
                for t in range(R // 128):
                    ptr = ps.tile([D, D], F32, tag="pxw", name="ptr", bufs=4)
                    nc.tensor.transpose(ptr[:], hn[:, bass.ts(t, D)], id_sb[:])
                    if t % 2 == 0:
                        nc.scalar.copy(out_sb[:, bass.ts(t, D)], ptr[:])
                    else:
                        nc.vector.tensor_copy(out_sb[:, bass.ts(t, D)], ptr[:])
                out_ap = out.ap().rearrange("(t p) f -> p t f", p=128)
                nc.sync.dma_start(out_ap, out_sb[:].rearrange(
                    "p (t f) -> p t f", f=D))

    nc.compile()
    n = _dedupe_ldweights(nc.m)
    log_n = n  # noqa: F841  (kept for debugging)
    return nc


def _ldw_sig(ins):
    return (repr(ins.ins[0]), repr(ins.perf_mode), repr(ins.is_transpose),
            repr(ins.tile_position), repr(ins.tile_size))


def _dedupe_ldweights(m):
    """Drop back-to-back InstLdweights that reload identical weights.

    bacc emits one LDWEIGHTS per matmul; the 4 same-weight matmuls per
    k-chunk then reload the PE array 3 extra times, serializing the MM
    stream. Dupes carry no sync_info, so removal is safe; any transpose
    or differing load resets the tracked signature.
    """
    removed = 0
    for f in m.functions:
        for bb in f.blocks:
            last_sig = None
            keep = []
            for ins in bb.instructions:
                tn = type(ins).__name__
                if tn == "InstLdweights":
                    si = ins.sync_info
                    clean = si is None or (not si.on_wait and not si.on_update)
                    sig = _ldw_sig(ins)
                    if clean and sig == last_sig:
                        removed += 1
                        continue
                    last_sig = sig
                elif tn == "InstMatmult" and ins.is_transpose:
                    last_sig = None
                keep.append(ins)
            bb.instructions[:] = keep
    return removed


_CACHED = {}


def _get_program():
    if "nc" not in _CACHED:
        _CACHED["nc"] = build_program()
    return _CACHED["nc"]


def _make_in_maps(x, A, W, b, gamma, beta):
    import ml_dtypes

    x = np.ascontiguousarray(np.asarray(x, dtype=np.float32))
    A = np.asarray(A, dtype=np.float32)
    W = np.ascontiguousarray(np.asarray(W, dtype=np.float32))
    b = np.asarray(b, dtype=np.float32).reshape(D, 1)
    gamma = np.asarray(gamma, dtype=np.float32).reshape(D, 1)
    beta = np.asarray(beta, dtype=np.float32).reshape(D, 1)
    ident = np.eye(D, dtype=np.float32)
    bf16 = ml_dtypes.bfloat16
    xt = np.ascontiguousarray(x.T)
    if A_MODE == "bf16":
        xt = xt.astype(bf16)
    elif A_MODE == "f16":
        xt = xt.astype(np.float16)
    common = {"xt": xt, "w": W, "bvec": b, "gam": gamma, "bet": beta,
              "ident": ident}
    in_maps = []
    for j in range(NCORES):
        at_j = np.ascontiguousarray(A[j * R:(j + 1) * R, :].T)
        m = dict(common)
        if A_MODE in ("bf16", "f16"):
            m["at"] = at_j.astype(bf16 if A_MODE == "bf16" else np.float16)
        elif A_MODE == "bf16x2":
            hi = at_j.astype(bf16)
            m["at"] = hi
            m["atlo"] = (at_j - hi.astype(np.float32)).astype(bf16)
        else:
            m["at"] = at_j
        in_maps.append(m)
    return in_maps


def run(x, A, W, b, gamma, beta, trace=False):
    nc = _get_program()
    in_maps = _make_in_maps(x, A, W, b, gamma, beta)
    res = run_bass_kernel_spmd(nc, in_maps, core_ids=list(range(NCORES)),
                               trace=trace)
    shards = [res.results[j]["out"] for j in range(NCORES)]
    full = np.concatenate(shards, axis=0)
    return full, res


def kernel(x, A, W, b, gamma, beta):
    full, _ = run(x, A, W, b, gamma, beta, trace=False)
    return full



# revision 4
# speedup vs baseline: 1.2356x; 1.2356x over previous
"""GCN block kernel for Trainium2 (8 NeuronCores, SPMD) — fp8 A-stream.

Computes: h = A @ (x @ W) + b; BatchNorm1d(train, biased var); LeakyReLU(0.2)
  x: [16384, 128] f32, A: [16384, 16384] f32, W: [128, 128], b/gamma/beta: [128]

Strategy (row-shard over output nodes, 8 cores x 2048 rows):
  - Associativity: h = (A @ x) @ W, so the big contraction streams A against
    x chunks (stationary, f16) instead of XW — no XW phase at all.
  - BatchNorm is invariant to any per-feature constant added to h, so both
    the bias b and the mean shift from centering A cancel exactly. The host
    sends at = 16*(A_shard^T - 0.5) in fp8 E3M4 (4 mantissa bits; centering
    halves magnitudes -> ~2x finer quantization; measured end-to-end
    rel_err ~1.05e-2 vs the 2e-2 gate).
  - fp8 halves the DMA stream to ~33.5 MB/core (~94 us at 358 GB/s/core);
    PE does 128x2048x... = 262144 column-pushes ~110 us warm — balanced.
  - g = (16(A-0.5)) @ x accumulates in PSUM over 128 k-chunks (mixed-dtype
    matmul: f16 stationary x-chunk, f8e3 moving A tile). Then g/16 cast to
    f16, h^T = W^T-free matmuls (W f16 stationary), BN stats via accum_out +
    DVE reduce, 8-core AllReduce of [128, 2], Prelu affine, PE-transpose to
    natural layout, DMA out.
  - A post-compile pass strips redundant per-matmul LDWEIGHTS reloads.
  - A dummy AllReduce early in the program absorbs the cold ncfw trigger
    cost so the tail stats AllReduce starts promptly.
"""

import numpy as np

import concourse.bass as bass
import concourse.bacc as bacc
import concourse.mybir as mybir
import concourse.tile as tile
from concourse.bass_utils import run_bass_kernel_spmd

N = 16384
D = 128
NCORES = 8
R = N // NCORES          # 2048 rows per core
KCH = N // 128           # 128 k-chunks
CPD = 8                  # k-chunks per at DMA (2 MiB per dma_start)
NGRP = KCH // CPD
EPS = 1e-5
NEG_SLOPE = 0.2
A_SCALE = 16.0           # at = A_SCALE * (A^T - 0.5), in [-8, 8] for E3M4

F32 = mybir.dt.float32
F16 = mybir.dt.float16
F8E3 = mybir.dt.float8e3


def build_program():
    nc = bacc.Bacc("TRN2", target_bir_lowering=False, debug=False,
                   num_devices=NCORES)

    at = nc.dram_tensor("at", [N, R], F8E3, kind="ExternalInput")
    # xt[p, c*D + d] = x[c*128 + p, d]  (k-chunk-tiled natural x, f16)
    xt = nc.dram_tensor("xt", [128, KCH * D], F16, kind="ExternalInput")
    w = nc.dram_tensor("w", [D, D], F32, kind="ExternalInput")
    gam = nc.dram_tensor("gam", [D, 1], F32, kind="ExternalInput")
    bet = nc.dram_tensor("bet", [D, 1], F32, kind="ExternalInput")
    ident = nc.dram_tensor("ident", [D, D], F32, kind="ExternalInput")
    out = nc.dram_tensor("out", [R, D], F32, kind="ExternalOutput")

    with tile.TileContext(nc) as tc:
        with (
            tc.tile_pool(name="const", bufs=1) as cpool,
            tc.tile_pool(name="xt", bufs=1) as xpool,
            tc.tile_pool(name="at", bufs=4) as atpool,
            tc.tile_pool(name="work", bufs=1) as wpool,
            tc.tile_pool(name="psum_g", bufs=1, space="PSUM") as pg,
            tc.tile_pool(name="psum_h", bufs=1, space="PSUM") as ph,
            tc.tile_pool(name="dram", bufs=1, space="DRAM") as dpool,
        ):
            # ---- xt chunks first (gates the first matmuls) on scalar q ----
            XSPL = 4
            XCW = KCH * D // XSPL  # columns per xt chunk tile
            xts = []
            for c in range(XSPL):
                t = xpool.tile([128, XCW], F16, tag="xt", bufs=XSPL)
                nc.scalar.dma_start(t[:], xt[:, bass.ts(c, XCW)])
                xts.append(t)

            def xchunk(k):  # [128, 128] f16 stationary operand for k-chunk k
                c, r = divmod(k * D, XCW)
                return xts[c][:, r:r + D]

            # ---- constants / params ----
            w_sb = cpool.tile([D, D], F32)
            nc.sync.dma_start(w_sb[:], w[:])
            id_sb = cpool.tile([D, D], F32)
            nc.sync.dma_start(id_sb[:], ident[:])
            gam_sb = cpool.tile([D, 1], F32)
            nc.sync.dma_start(gam_sb[:], gam[:])
            bet_sb = cpool.tile([D, 1], F32)
            nc.sync.dma_start(bet_sb[:], bet[:])
            zero_sb = cpool.tile([D, 1], F32)
            nc.gpsimd.memset(zero_sb[:], 0.0)
            eps_sb = cpool.tile([D, 1], F32)
            nc.gpsimd.memset(eps_sb[:], EPS)
            w16_sb = cpool.tile([D, D], F16)
            nc.vector.tensor_copy(w16_sb[:], w_sb[:])

            # warm up the collective path early so the real stats
            # AllReduce at the tail doesn't pay the cold ncfw trigger cost
            warm_in = dpool.tile([D, 2], F32, name="warm_in")
            warm_out = dpool.tile([D, 2], F32, addr_space="Shared",
                                  name="warm_out")
            warm_sb = cpool.tile([D, 2], F32, name="warm_sb")
            nc.gpsimd.memset(warm_sb[:], 0.0)
            nc.gpsimd.dma_start(warm_in[:], warm_sb[:])
            nc.gpsimd.collective_compute(
                "AllReduce", mybir.AluOpType.add,
                replica_groups=[list(range(NCORES))],
                ins=[warm_in.opt()], outs=[warm_out.opt()])

            # ---- main: g^T[d, n] = sum_k at[k, n] * x[k, d] over 128 chunks
            at_r = at.ap().rearrange("(c p) j -> c p j", p=128)
            psum_g = pg.tile([D, R], F32)  # 4 PSUM banks
            for i in range(NGRP):
                at_t = atpool.tile([128, CPD, R], F8E3, tag="at")
                dma_eng = nc.sync if i % 2 == 0 else nc.scalar
                dma_eng.dma_start(
                    at_t[:], at_r[CPD * i:CPD * (i + 1)].transpose([1, 0, 2]))
                for a in range(CPD):
                    k = CPD * i + a
                    for s in range(R // 512):
                        nc.tensor.matmul(
                            psum_g[:, bass.ts(s, 512)],
                            xchunk(k),
                            at_t[:, a, bass.ts(s, 512)],
                            start=(k == 0), stop=(k == KCH - 1),
                        )

            # ---- g -> f16 with 1/A_SCALE folded in ----
            g16 = wpool.tile([D, R], F16)
            for s in range(4):
                nc.scalar.activation(
                    g16[:, bass.ts(s, 512)], psum_g[:, bass.ts(s, 512)],
                    mybir.ActivationFunctionType.Identity,
                    bias=zero_sb[:], scale=1.0 / A_SCALE)

            # ---- h^T[f, n] = sum_d W[d, f] * g16[d, n] ----
            psum_h = ph.tile([D, R], F32)  # 4 PSUM banks
            for s in range(4):
                nc.tensor.matmul(
                    psum_h[:, bass.ts(s, 512)], w16_sb[:],
                    g16[:, bass.ts(s, 512)], start=True, stop=True)

            # ---- stats: sums of h (DVE) and h^2 (ACT accum) ----
            sums = wpool.tile([D, 8], F32)
            sq_sb = atpool.tile([128, R], F32, tag="scr", bufs=2,
                                name="sq_sb")
            for s in range(4):
                nc.scalar.activation(
                    sq_sb[:, bass.ts(s, 512)], psum_h[:, bass.ts(s, 512)],
                    mybir.ActivationFunctionType.Square,
                    bias=zero_sb[:], accum_out=sums[:, 4 + s:5 + s])
            for s in range(4):
                nc.vector.reduce_sum(sums[:, s:s + 1],
                                     psum_h[:, bass.ts(s, 512)],
                                     axis=mybir.AxisListType.X)

            stats = wpool.tile([D, 2], F32)
            nc.vector.reduce_sum(stats[:, 0:1], sums[:, 0:4],
                                 axis=mybir.AxisListType.X)
            nc.vector.reduce_sum(stats[:, 1:2], sums[:, 4:8],
                                 axis=mybir.AxisListType.X)

            # ---- AllReduce of [128, 2] stats across 8 cores ----
            cc_in = dpool.tile([D, 2], F32)
            cc_out = dpool.tile([D, 2], F32, addr_space="Shared")
            nc.gpsimd.dma_start(cc_in[:], stats[:])
            nc.gpsimd.collective_compute(
                "AllReduce", mybir.AluOpType.add,
                replica_groups=[list(range(NCORES))],
                ins=[cc_in.opt()], outs=[cc_out.opt()])
            stats_g = wpool.tile([D, 2], F32)
            nc.gpsimd.dma_start(stats_g[:], cc_out[:])

            # ---- per-feature scale/shift (b cancels in BN; not used) ----
            mean = wpool.tile([D, 1], F32)
            nc.scalar.mul(mean[:], stats_g[:, 0:1], 1.0 / N)
            ex2 = wpool.tile([D, 1], F32)
            nc.scalar.mul(ex2[:], stats_g[:, 1:2], 1.0 / N)
            msq = wpool.tile([D, 1], F32)
            nc.vector.tensor_mul(msq[:], mean[:], mean[:])
            var = wpool.tile([D, 1], F32)
            nc.vector.tensor_sub(var[:], ex2[:], msq[:])
            std = wpool.tile([D, 1], F32)
            nc.scalar.activation(std[:], var[:],
                                 mybir.ActivationFunctionType.Sqrt,
                                 bias=eps_sb[:])
            istd = wpool.tile([D, 1], F32)
            nc.vector.reciprocal(istd[:], std[:])
            scl = wpool.tile([D, 1], F32)
            nc.vector.tensor_mul(scl[:], gam_sb[:], istd[:])
            tmp = wpool.tile([D, 1], F32)
            nc.vector.tensor_mul(tmp[:], mean[:], scl[:])
            shf = wpool.tile([D, 1], F32)
            nc.vector.tensor_sub(shf[:], bet_sb[:], tmp[:])

            # ---- y = LeakyReLU(scl*h + shf), still [f, n] ----
            hn = atpool.tile([128, R], F32, tag="scr", bufs=2,
                             name="hn_sb")[:]
            for s in range(4):
                nc.scalar.activation(
                    hn[:, bass.ts(s, 512)], psum_h[:, bass.ts(s, 512)],
                    mybir.ActivationFunctionType.Prelu,
                    bias=shf[:], scale=scl[:], alpha=NEG_SLOPE)

            # ---- transpose to [n, f] (reusing psum_g banks) and store ----
            out_sb = atpool.tile([128, R], F32, tag="scr", bufs=2,
                                 name="out_t")
            for t in range(R // 128):
                ptr = psum_g[:, bass.ts(t % 8, D)]
                nc.tensor.transpose(ptr, hn[:, bass.ts(t, D)], id_sb[:])
                if t % 2 == 0:
                    nc.scalar.copy(out_sb[:, bass.ts(t, D)], ptr)
                else:
                    nc.vector.tensor_copy(out_sb[:, bass.ts(t, D)], ptr)
            out_ap = out.ap().rearrange("(t p) f -> p t f", p=128)
            nc.sync.dma_start(out_ap, out_sb[:].rearrange(
                "p (t f) -> p t f", f=D))

    nc.compile()
    _dedupe_ldweights(nc.m)
    return nc


def _ldw_sig(ins):
    return (repr(ins.ins[0]), repr(ins.perf_mode), repr(ins.is_transpose),
            repr(ins.tile_position), repr(ins.tile_size))


def _dedupe_ldweights(m):
    """Drop back-to-back InstLdweights that reload identical weights."""
    removed = 0
    for f in m.functions:
        for bb in f.blocks:
            last_sig = None
            keep = []
            for ins in bb.instructions:
                tn = type(ins).__name__
                if tn == "InstLdweights":
                    si = ins.sync_info
                    clean = si is None or (not si.on_wait and not si.on_update)
                    sig = _ldw_sig(ins)
                    if clean and sig == last_sig:
                        removed += 1
                        continue
                    last_sig = sig
                elif tn == "InstMatmult" and ins.is_transpose:
                    last_sig = None
                keep.append(ins)
            bb.instructions[:] = keep
    return removed


_CACHED = {}


def _get_program():
    if "nc" not in _CACHED:
        _CACHED["nc"] = build_program()
    return _CACHED["nc"]


def _make_in_maps(x, A, W, b, gamma, beta):
    import ml_dtypes

    x = np.asarray(x, dtype=np.float32)
    A = np.asarray(A, dtype=np.float32)
    W = np.ascontiguousarray(np.asarray(W, dtype=np.float32))
    gamma = np.asarray(gamma, dtype=np.float32).reshape(D, 1)
    beta = np.asarray(beta, dtype=np.float32).reshape(D, 1)
    ident = np.eye(D, dtype=np.float32)

    # xt[p, c*D + d] = x[c*128 + p, d]
    xt = np.ascontiguousarray(
        x.astype(np.float16).reshape(KCH, 128, D).transpose(1, 0, 2)
    ).reshape(128, KCH * D)

    common = {"xt": xt, "w": W, "gam": gamma, "bet": beta, "ident": ident}
    in_maps = []
    for j in range(NCORES):
        at_j = np.ascontiguousarray(A[j * R:(j + 1) * R, :].T)
        at_j = ((at_j - np.float32(0.5)) * np.float32(A_SCALE)).astype(
            ml_dtypes.float8_e3m4)
        m = dict(common)
        m["at"] = at_j
        in_maps.append(m)
    return in_maps


def run(x, A, W, b, gamma, beta, trace=False):
    nc = _get_program()
    in_maps = _make_in_maps(x, A, W, b, gamma, beta)
    res = run_bass_kernel_spmd(nc, in_maps, core_ids=list(range(NCORES)),
                               trace=trace)
    shards = [res.results[j]["out"] for j in range(NCORES)]
    full = np.concatenate(shards, axis=0)
    return full, res


def kernel(x, A, W, b, gamma, beta):
    full, _ = run(x, A, W, b, gamma, beta, trace=False)
    return full
